# revision 18
# baseline (speedup 1.0000x reference)
"""Multi-head attention (B=2, L=2048, D=1024, H=16, d=64) on 8 TRN2 NeuronCores.

Sharding: core c -> batch b = c // 4, head group g = c % 4 (heads 4g..4g+3).
Each core computes Q/K/V projections for its 4 heads (tensor-parallel column
split), attention, and a row-parallel partial of the output projection.
Host gathers: out[b] = sum of the 4 partial outputs of batch b's cores.

Device layouts (per core):
  xqT/xkT/xvT [1024, 2048]  : input_X[b].T           (m on partitions)
  wqT/wkT/wvT [1024, 256]   : W_X[g-block].T         (wqT pre-scaled by 1/8)
  bq/bk  [1, 256] per-partition bias chunks, bv [1, 256], bo [1, 1024]
  woT [256, 1024]           : W_out[:, g-block].T
  maskT [2048, 2048] bf16   : (1 - mask[b]).T        (multiplicative, [k, q])
  outT [1024, 2048] f32     : partial out[b].T

On-chip: Q.T/K.T [256, 2048] resident; scores computed transposed S.T[k, q] =
K.T(chunk) x Q.T so exp (ScalarE) + mask-mult (VectorE) happen in [k_part, q]
layout; attn@V uses lhsT = [V | ones] giving out.T rows 0..63 and the softmax
denominator in row 64; normalize via reciprocal + ones-broadcast matmul.
"""

import sys

sys.path.insert(0, "/opt/trn_rl_repo")

import ml_dtypes
import numpy as np

D_MODEL = 1024
NUM_HEADS = 16
DK = 64
B, L = 2, 2048
NCORES = 8
GROUPS = 4                  # head groups (cores per batch)
HPC = NUM_HEADS // GROUPS   # heads per core = 4
DH = HPC * DK               # 256
MC = D_MODEL // 128         # 8 m-chunks
KC = L // 128               # 16 k-chunks
TQ = 512                    # query tile
NQT = L // TQ               # 4 query tiles


def build_nc():
    import concourse.mybir as mybir
    import concourse.tile as tile
    from concourse import bacc

    F32 = mybir.dt.float32
    F32R = mybir.dt.float32r
    BF16 = mybir.dt.bfloat16
    Exp = mybir.ActivationFunctionType.Exp
    Copy = mybir.ActivationFunctionType.Copy

    nc = bacc.Bacc(None, target_bir_lowering=False)

    xqT = nc.declare_dram_parameter("xqT", [D_MODEL, L], F32R, isOutput=False)
    xkT = nc.declare_dram_parameter("xkT", [D_MODEL, L], F32R, isOutput=False)
    xvT = nc.declare_dram_parameter("xvT", [D_MODEL, L], F32R, isOutput=False)
    wqT = nc.declare_dram_parameter("wqT", [D_MODEL, DH], F32R, isOutput=False)
    wkT = nc.declare_dram_parameter("wkT", [D_MODEL, DH], F32R, isOutput=False)
    wvT = nc.declare_dram_parameter("wvT", [D_MODEL, DH], F32R, isOutput=False)
    bq = nc.declare_dram_parameter("bq", [1, DH], F32, isOutput=False)
    bk = nc.declare_dram_parameter("bk", [1, DH], F32, isOutput=False)
    bv = nc.declare_dram_parameter("bv", [1, DH], F32R, isOutput=False)
    woT = nc.declare_dram_parameter("woT", [DH, D_MODEL], F32R, isOutput=False)
    bo = nc.declare_dram_parameter("bo", [1, D_MODEL], F32R, isOutput=False)
    onesd = nc.declare_dram_parameter("onesd", [1, TQ], F32R, isOutput=False)
    maskT = nc.declare_dram_parameter("maskT", [L, L], BF16, isOutput=False)
    outT = nc.declare_dram_parameter("outT", [D_MODEL, L], F32, isOutput=True)

    def r(ap):  # operands are already f32r
        return ap

    with tile.TileContext(nc) as tc:
        with (
            tc.tile_pool(name="wres", bufs=1) as wres,
            tc.tile_pool(name="qkv", bufs=1) as qkv,
        ):
            # resident weights / constants
            wq_sb = wres.tile([128, MC, DH], F32R, tag="wq")
            wk_sb = wres.tile([128, MC, DH], F32R, tag="wk")
            wv_sb = wres.tile([128, MC, DH], F32R, tag="wv")
            wo_sb = wres.tile([64, HPC, D_MODEL], F32R, tag="wo")
            bq_sb = wres.tile([128, DH // 128], F32, tag="bq")
            bk_sb = wres.tile([128, DH // 128], F32, tag="bk")
            bv_sb = wres.tile([1, DH], F32R, tag="bv")
            bo_sb = wres.tile([1, D_MODEL], F32R, tag="bo")
            ones_sb = wres.tile([1, TQ], F32R, tag="ones")
            ones_bc = wres.tile([128, DK], F32R, tag="ones_bc")

            nc.sync.dma_start(wq_sb, wqT.rearrange("(c p) d -> p c d", p=128))
            nc.sync.dma_start(wk_sb, wkT.rearrange("(c p) d -> p c d", p=128))
            nc.sync.dma_start(wv_sb, wvT.rearrange("(c p) d -> p c d", p=128))
            nc.sync.dma_start(wo_sb, woT.rearrange("(h p) m -> p h m", p=64))
            nc.sync.dma_start(bq_sb, bq.rearrange("x (c p) -> p (x c)", p=128))
            nc.sync.dma_start(bk_sb, bk.rearrange("x (c p) -> p (x c)", p=128))
            nc.sync.dma_start(bv_sb, bv[:])
            nc.sync.dma_start(bo_sb, bo[:])
            nc.sync.dma_start(ones_sb, onesd[:])
            nc.sync.dma_start(ones_bc, onesd[0:1, 0:DK].to_broadcast([128, DK]))

            # resident Q.T / K.T (f32) and V (bf16, with ones column)
            qt_sb = qkv.tile([128, DH // 128, L], F32R, tag="qt")
            kt_sb = qkv.tile([128, DH // 128, L], F32R, tag="kt")
            v_sb = qkv.tile([128, KC, HPC, DK + 1], BF16, tag="v")
            nc.vector.memset(v_sb[:, :, :, DK : DK + 1], 1.0)

            # ---- Phase A: projections ----
            with (
                tc.tile_pool(name="xs", bufs=MC) as xs,
                tc.tile_pool(name="psA", bufs=2, space="PSUM") as psA,
            ):
                # Q.T and K.T: [256, 2048] = wT.T @ xT  (+ bias via ACT copy)
                for name, xT, w_sb, b_sb, dst in (
                    ("q", xqT, wq_sb, bq_sb, qt_sb),
                    ("k", xkT, wk_sb, bk_sb, kt_sb),
                ):
                    pss = [
                        psA.tile([128, L], F32, tag="pa", name=f"ps_{name}{md}")
                        for md in range(DH // 128)
                    ]
                    for mc in range(MC):
                        x_t = xs.tile([128, L], F32R, tag="x", name=f"x_{name}_{mc}")
                        nc.sync.dma_start(x_t, xT[mc * 128 : (mc + 1) * 128, :])
                        for md in range(DH // 128):
                            for nq in range(L // 512):
                                nc.tensor.matmul(
                                    pss[md][:, nq * 512 : (nq + 1) * 512],
                                    r(w_sb[:, mc, md * 128 : (md + 1) * 128]),
                                    r(x_t[:, nq * 512 : (nq + 1) * 512]),
                                    start=(mc == 0),
                                    stop=(mc == MC - 1),
                                )
                    for md in range(DH // 128):
                        nc.scalar.activation(
                            dst[:, md, :],
                            pss[md],
                            mybir.ActivationFunctionType.Identity,
                            bias=b_sb[:, md : md + 1],
                        )
                # V: [2048, 256] = xvT.T @ wvT + ones.T @ bv
                xv_ts = []
                for mc in range(MC):
                    xv_t = xs.tile([128, L], F32R, tag="x", name=f"x_v_{mc}")
                    nc.sync.dma_start(xv_t, xvT[mc * 128 : (mc + 1) * 128, :])
                    xv_ts.append(xv_t)
                for lc in range(KC):
                    psv = psA.tile([128, DH], F32, tag="pa", name=f"ps_v{lc}")
                    nc.tensor.matmul(
                        psv, r(ones_sb[:, 0:128]), r(bv_sb), start=True, stop=False
                    )
                    for mc in range(MC):
                        nc.tensor.matmul(
                            psv,
                            r(xv_ts[mc][:, lc * 128 : (lc + 1) * 128]),
                            r(wv_sb[:, mc, :]),
                            start=False,
                            stop=(mc == MC - 1),
                        )
                    nc.vector.tensor_copy(
                        v_sb[:, lc, :, 0:DK],
                        psv.rearrange("p (h d) -> p h d", h=HPC),
                    )

            # ---- Phase B: attention + output projection ----
            with (
                tc.tile_pool(name="mq", bufs=2) as mqp,
                tc.tile_pool(name="pts", bufs=KC + 3) as ptp,
                tc.tile_pool(name="ats", bufs=2 * HPC) as atp,
                tc.tile_pool(name="rcs", bufs=2) as rcp,
                tc.tile_pool(name="outs", bufs=3) as outp,
                tc.tile_pool(name="psB", bufs=1, space="PSUM") as psB,
            ):
                for qt in range(NQT):
                    q0 = qt * TQ
                    mask_q = mqp.tile([128, KC, TQ], BF16, tag="mq")
                    nc.sync.dma_start(
                        mask_q,
                        maskT[:, q0 : q0 + TQ].rearrange("(c p) q -> p c q", p=128),
                    )
                    ats = []
                    for h in range(HPC):
                        hoff = (h % 2) * 64
                        hc = h // 2
                        pts = []
                        for kc in range(KC):
                            ps_s = psB.tile([128, TQ], F32, tag="s", bufs=3)
                            nc.tensor.matmul(
                                ps_s,
                                r(kt_sb[hoff : hoff + 64, hc, kc * 128 : (kc + 1) * 128]),
                                r(qt_sb[hoff : hoff + 64, hc, q0 : q0 + TQ]),
                                start=True,
                                stop=True,
                            )
                            pt = ptp.tile([128, TQ], BF16, tag="pt")
                            nc.scalar.activation(pt, ps_s, Exp)
                            nc.vector.tensor_mul(pt, pt, mask_q[:, kc, :])
                            pts.append(pt)
                        ps_o = psB.tile([DK + 1, TQ], F32, tag="o", bufs=2)
                        for kc in range(KC):
                            nc.tensor.matmul(
                                ps_o,
                                v_sb[:, kc, h, :],
                                pts[kc],
                                start=(kc == 0),
                                stop=(kc == KC - 1),
                            )
                        rc = rcp.tile([DK + 1, TQ], F32R, tag="rc")
                        with nc.allow_low_precision(reason="f32r rounding for PE"):
                            nc.vector.reciprocal(
                                rc[DK : DK + 1, :], ps_o[DK : DK + 1, :]
                            )
                        ps_r = psB.tile([DK, TQ], F32, tag="r", bufs=1)
                        nc.tensor.matmul(
                            ps_r,
                            r(ones_bc[64:65, 0:DK]),
                            r(rc[DK : DK + 1, :]),
                            start=True,
                            stop=True,
                        )
                        rcb = rcp.tile([DK, TQ], F32, tag="rcb")
                        nc.vector.tensor_copy(rcb, ps_r)
                        at = atp.tile([DK, TQ], F32R, tag="at")
                        with nc.allow_low_precision(reason="f32r rounding for PE"):
                            nc.vector.tensor_mul(at, ps_o[0:DK, :], rcb)
                        ats.append(at)
                    for mc in range(MC):
                        ps_out = psB.tile([128, TQ], F32, tag="po", bufs=2)
                        m0 = mc * 128
                        nc.tensor.matmul(
                            ps_out,
                            r(bo_sb[:, m0 : m0 + 128]),
                            r(ones_sb),
                            start=True,
                            stop=False,
                        )
                        for h in range(HPC):
                            nc.tensor.matmul(
                                ps_out,
                                r(wo_sb[:, h, m0 : m0 + 128]),
                                r(ats[h]),
                                start=False,
                                stop=(h == HPC - 1),
                            )
                        o_t = outp.tile([128, TQ], F32, tag="ot")
                        nc.vector.tensor_copy(o_t, ps_out)
                        nc.sync.dma_start(outT[m0 : m0 + 128, q0 : q0 + TQ], o_t)

    nc.compile()
    return nc


def round_fp32r(a):
    """Round fp32 to fp32r (11-bit mantissa, low 12 bits zero), RNE."""
    u = np.ascontiguousarray(a, dtype=np.float32).view(np.uint32)
    r = u + np.uint32(0x7FF) + ((u >> np.uint32(12)) & np.uint32(1))
    r &= np.uint32(0xFFFFF000)
    return r.view(np.float32)


def make_in_maps(input_Q, input_K, input_V, attn_mask, W_Q, b_Q, W_K, b_K,
                 W_V, b_V, W_out, b_out):
    bf16 = ml_dtypes.bfloat16
    scale = np.float32(1.0 / np.sqrt(DK))
    input_Q = np.asarray(input_Q, dtype=np.float32)
    input_K = np.asarray(input_K, dtype=np.float32)
    input_V = np.asarray(input_V, dtype=np.float32)
    attn_mask = np.asarray(attn_mask)
    W_Q = np.asarray(W_Q, dtype=np.float32)
    W_K = np.asarray(W_K, dtype=np.float32)
    W_V = np.asarray(W_V, dtype=np.float32)
    W_out = np.asarray(W_out, dtype=np.float32)
    b_Q = np.asarray(b_Q, dtype=np.float32)
    b_K = np.asarray(b_K, dtype=np.float32)
    b_V = np.asarray(b_V, dtype=np.float32)
    b_out = np.asarray(b_out, dtype=np.float32)
    in_maps = []
    for c in range(NCORES):
        b = c // GROUPS
        g = c % GROUPS
        r0 = g * DH
        maskT = np.ascontiguousarray(
            (1.0 - attn_mask[b].astype(np.float32)).T.astype(bf16)
        )
        in_maps.append({
            "xqT": round_fp32r(input_Q[b].T),
            "xkT": round_fp32r(input_K[b].T),
            "xvT": round_fp32r(input_V[b].T),
            "wqT": round_fp32r(W_Q[r0 : r0 + DH].T * scale),
            "wkT": round_fp32r(W_K[r0 : r0 + DH].T),
            "wvT": round_fp32r(W_V[r0 : r0 + DH].T),
            "bq": np.ascontiguousarray(b_Q[None, r0 : r0 + DH] * scale),
            "bk": np.ascontiguousarray(b_K[None, r0 : r0 + DH]),
            "bv": round_fp32r(b_V[None, r0 : r0 + DH]),
            "woT": round_fp32r(W_out[:, r0 : r0 + DH].T),
            "bo": round_fp32r(
                (b_out if g == 0 else np.zeros_like(b_out))[None, :]
            ),
            "onesd": np.ones((1, TQ), dtype=np.float32),
            "maskT": maskT,
        })
    return in_maps


def gather(results):
    out = np.zeros((B, L, D_MODEL), dtype=np.float32)
    for c in range(NCORES):
        out[c // GROUPS] += results[c]["outT"].T
    return out


def _ensure_ntff_hook():
    """The container's antenv stub lacks axon_hooks; recreate it so
    run_bass_kernel_spmd(trace=True) can reach the NTFF profiler."""
    import types

    try:
        from antenv.axon_hooks import get_axon_ntff_profile_hook  # noqa: F401
        return
    except ImportError:
        pass
    mod = types.ModuleType("antenv.axon_hooks")
    _hook = [None]
    mod.set_axon_ntff_profile_hook = lambda h: _hook.__setitem__(0, h)
    mod.get_axon_ntff_profile_hook = lambda: _hook[0]
    sys.modules["antenv.axon_hooks"] = mod
    import antenv

    antenv.axon_hooks = mod
    from trn_agent_boot.trn_boot import _ntff_profile_via_ctypes

    mod.set_axon_ntff_profile_hook(
        _ntff_profile_via_ctypes("/opt/axon/libaxon_pjrt.so")
    )


def run(inputs, trace=False):
    from concourse.bass_utils import run_bass_kernel_spmd

    if trace:
        _ensure_ntff_hook()
    nc = build_nc()
    in_maps = make_in_maps(**inputs)
    res = run_bass_kernel_spmd(
        nc, in_maps, core_ids=list(range(NCORES)), trace=trace
    )
    return gather(res.results), res


def kernel(**inputs):
    out, _ = run(inputs, trace=False)
    return out
